# revision 10
# baseline (speedup 1.0000x reference)
"""Multi-head attention (RoPE, causal) on 8 TRN2 NeuronCores.

Sharding: core c -> batch b = c//2, head-group g = c%2 (8 of 16 heads).

v4: chunk-outer pipeline.  Attention runs over 512-query chunks; q/k/v
projections for chunk c+1 and the o-projection for chunk c-1 are emitted
as filler between attention groups so the PE never idles (HAM stays at
K=8/8).  Scores for a head PAIR are row-tiled (64-contract tiles at
partitions 0-63 / 64-127) so two heads' score matmuls run concurrently.
q/k stay SBUF-resident (SBUF->SBUF DMA re-layout), y staged via DRAM.
"""

import numpy as np
from contextlib import ExitStack
from functools import partial
from collections import deque

import concourse.bacc as bacc
import concourse.bass as bass
import concourse.mybir as mybir
import concourse.tile as tile
from concourse.bass_utils import run_bass_kernel_spmd
from concourse.masks import make_upper_triangular

F32 = mybir.dt.float32
F32R = mybir.dt.float32r
AF = mybir.ActivationFunctionType

D = 1024
S = 2048
NH = 16
DK = 64
HPC = 8          # heads per core
HD = HPC * DK    # 512
NCORES = 8
THETA = 10000.0

NS = S // 128    # 16 key tiles
NC_ = 4          # 512-query chunks
NK = D // 128    # 8 contraction tiles
CW = 512         # chunk width

_CACHE = {}


def _build_nc():
    nc = bacc.Bacc(None, target_bir_lowering=False)

    XT = nc.dram_tensor("XT", [D, S], F32R, kind="ExternalInput")
    WQ = nc.dram_tensor("WQ", [D, HD], F32R, kind="ExternalInput")
    WK = nc.dram_tensor("WK", [D, HD], F32R, kind="ExternalInput")
    WV = nc.dram_tensor("WV", [D, HD], F32R, kind="ExternalInput")
    OC = nc.dram_tensor("OC", [HD, D], F32R, kind="ExternalInput")
    COS = nc.dram_tensor("COS", [128, S], F32, kind="ExternalInput")
    SIN = nc.dram_tensor("SIN", [128, S], F32, kind="ExternalInput")
    OT = nc.dram_tensor("OT", [D, S], F32, kind="ExternalOutput")
    YD = nc.dram_tensor("YD", [HD, S], F32R, kind="ExternalOutput")

    with tile.TileContext(nc) as tc, ExitStack() as ctx:
        const = ctx.enter_context(tc.tile_pool(name="const", bufs=1))
        dram = ctx.enter_context(tc.tile_pool(name="dram", bufs=1,
                                              space="DRAM"))
        kres = ctx.enter_context(tc.tile_pool(name="kres", bufs=1))
        psp = ctx.enter_context(tc.tile_pool(name="psp", bufs=1,
                                             space="PSUM"))
        xtp = ctx.enter_context(tc.tile_pool(name="xtp", bufs=1))
        wst = ctx.enter_context(tc.tile_pool(name="wst", bufs=3))
        wvp = ctx.enter_context(tc.tile_pool(name="wvp", bufs=1))
        ocp = ctx.enter_context(tc.tile_pool(name="ocp", bufs=1))
        csp = ctx.enter_context(tc.tile_pool(name="csp", bufs=2))
        qp = ctx.enter_context(tc.tile_pool(name="qp", bufs=8))
        tp = ctx.enter_context(tc.tile_pool(name="tp", bufs=4))
        qs = ctx.enter_context(tc.tile_pool(name="qs", bufs=3))
        ptp = ctx.enter_context(tc.tile_pool(name="ptp", bufs=3))
        osp = ctx.enter_context(tc.tile_pool(name="osp", bufs=2))
        yip = ctx.enter_context(tc.tile_pool(name="yip", bufs=4))

        # ---- constants ----
        # mD holds all diagonal masks as views: [0:384]=0, [384:512]=tri.
        # tri = mD[:,384:512]; mB = mD[:,256:512]; mC = mD[:,128:512]
        mD = const.tile([128, 512], F32, tag="mD")
        nc.vector.memset(mD[:, 0:384], 0.0)
        make_upper_triangular(nc, mD[:, 384:512], val=1.0, diag=True)
        tri = mD[:, 384:512]
        mB = mD[:, 256:512]
        mC = mD[:, 128:512]

        # ---- V (with interleaved ones column for the softmax denom) ----
        ones128 = const.tile([128, 128], F32, tag="ones128")
        nc.vector.memset(ones128, 1.0)
        vpall = kres.tile([128, NS, HPC * 65], F32R, tag="vpall")
        vones = vpall[:, :, :].rearrange("p j (h e) -> p j h e", e=65)
        nc.vector.tensor_copy(
            vones[:, :, :, 64],
            ones128[:, :].rearrange("p (j e) -> p j e", e=8))

        # ---- K pair tiles (SBUF resident) ----
        # pair P = heads (2P, 2P+1); rows 0-63 head 2P, 64-127 head 2P+1
        kt = [[kres.tile([128, CW], F32R, tag=f"kt{P}_{c}",
                         name=f"kt{P}_{c}") for c in range(NC_)]
              for P in range(4)]

        yd = YD

        # ---- resident weights (V and O only; WQ/WK are streamed) ----
        wv = []
        for k in range(NK):
            w = wvp.tile([128, HD], F32R, tag=f"wv{k}")
            nc.sync.dma_start(out=w, in_=WV[k * 128:(k + 1) * 128, :])
            wv.append(w)
        oct_sb = []
        for p in range(4):
            o_t = ocp.tile([128, D], F32R, tag=f"oct{p}")
            nc.sync.dma_start(out=o_t, in_=OC[p * 128:(p + 1) * 128, :])
            oct_sb.append(o_t)

        # ---- chunked x / cos / sin loads ----
        xtc = {}
        cs_t = {}

        def load_xt(c):
            t = xtp.tile([128, NK, CW], F32R, tag="xt")
            nc.sync.dma_start(
                out=t,
                in_=XT[:, c * CW:(c + 1) * CW].rearrange(
                    "(k r) s -> r k s", r=128))
            xtc[c] = t
            co = csp.tile([128, CW], F32, tag="cosc")
            nc.sync.dma_start(out=co, in_=COS[:, c * CW:(c + 1) * CW])
            si = csp.tile([128, CW], F32, tag="sinc")
            nc.sync.dma_start(out=si, in_=SIN[:, c * CW:(c + 1) * CW])
            cs_t[c] = (co, si)

        q_tiles = {}

        def qk_proj(tens, pg, c):
            """Project+rope chunk c of q or k for p-group pg (heads 4pg..4pg+3),
            scatter into the two pair tiles."""
            Wt = WQ if tens == "q" else WK
            w1 = wst.tile([128, NK, 128], F32R, tag="w", name="w1")
            nc.sync.dma_start(
                out=w1,
                in_=Wt[:, pg * 128:(pg + 1) * 128].rearrange(
                    "(k r) c -> r k c", r=128))
            w2 = wst.tile([128, NK, 128], F32R, tag="w", name="w2")
            nc.sync.dma_start(
                out=w2,
                in_=Wt[:, 256 + pg * 128:256 + (pg + 1) * 128].rearrange(
                    "(k r) c -> r k c", r=128))
            xt = xtc[c]
            cos_c, sin_c = cs_t[c]
            ps1 = psp.tile([128, CW], F32, tag="pab", bufs=2, name="ps1")
            for k in range(NK):
                nc.tensor.matmul(ps1, w1[:, k, :], xt[:, k, :],
                                 start=(k == 0), stop=(k == NK - 1))
            ps2 = psp.tile([128, CW], F32, tag="pab", bufs=2, name="ps2")
            for k in range(NK):
                nc.tensor.matmul(ps2, w2[:, k, :], xt[:, k, :],
                                 start=(k == 0), stop=(k == NK - 1))
            tA = tp.tile([128, CW], F32, tag="rt", name="tA")
            nc.vector.tensor_mul(tA, ps1, cos_c)
            tC = tp.tile([128, CW], F32, tag="rt", name="tC")
            nc.vector.tensor_mul(tC, ps1, sin_c)
            tB = tp.tile([128, CW], F32, tag="rt", name="tB")
            nc.vector.tensor_mul(tB, ps2, sin_c)
            tD = tp.tile([128, CW], F32, tag="rt", name="tD")
            nc.vector.tensor_mul(tD, ps2, cos_c)
            o1 = qs.tile([128, CW], F32R, tag="ro", name="o1")
            nc.vector.tensor_sub(o1, tA, tB)
            o2 = qs.tile([128, CW], F32R, tag="ro", name="o2")
            nc.vector.tensor_add(o2, tC, tD)
            for t in (0, 1):
                P = 2 * pg + t
                if tens == "q":
                    dst = qp.tile([128, CW], F32R, tag="q",
                                  name=f"qt{P}_{c}")
                    q_tiles[(P, c)] = dst
                else:
                    dst = kt[P][c]
                eng = nc.sync if t == 0 else nc.gpsimd
                eng.dma_start(out=dst[0:32, :], in_=o1[64 * t:64 * t + 32, :])
                eng.dma_start(out=dst[64:96, :],
                              in_=o1[64 * t + 32:64 * t + 64, :])
                eng.dma_start(out=dst[32:64, :], in_=o2[64 * t:64 * t + 32, :])
                eng.dma_start(out=dst[96:128, :],
                              in_=o2[64 * t + 32:64 * t + 64, :])

        def v_unit(c, jj):
            """V projection for key tile jj (in chunk c's column range)."""
            psv = psp.tile([128, CW], F32, tag="pab", bufs=2, name="psv")
            xs = slice((jj - 4 * c) * 128, (jj - 4 * c + 1) * 128)
            for k in range(NK):
                nc.tensor.matmul(psv, xtc[c][:, k, xs], wv[k],
                                 start=(k == 0), stop=(k == NK - 1))
            vslice = vpall[:, jj, :].rearrange("p (h e) -> p h e", e=65)
            if jj % 2 == 0:
                nc.scalar.copy(vslice[:, :, 0:64],
                               psv.rearrange("p (h e) -> p h e", e=64))
            else:
                nc.vector.tensor_copy(vslice[:, :, 0:64],
                                      psv.rearrange("p (h e) -> p h e", e=64))

        def oproj_yin(c):
            yin = []
            for p in range(4):
                y_t = yip.tile([128, CW], F32R, tag="yin")
                nc.sync.dma_start(
                    out=y_t, in_=yd[p * 128:(p + 1) * 128,
                                    c * CW:(c + 1) * CW])
                yin.append(y_t)
            oproj_yin_t[c] = yin

        oproj_yin_t = {}

        def oproj_dt(c, dt):
            yin = oproj_yin_t[c]
            pd = psp.tile([128, CW], F32, tag="pab", bufs=2, name="pd")
            for p in range(4):
                nc.tensor.matmul(pd, oct_sb[p][:, dt * 128:(dt + 1) * 128],
                                 yin[p], start=(p == 0), stop=(p == 3))
            o_s = osp.tile([128, CW], F32, tag="os")
            if dt % 2 == 0:
                nc.scalar.copy(o_s, pd)
            else:
                nc.vector.tensor_copy(o_s, pd)
            nc.gpsimd.dma_start(
                out=OT[dt * 128:(dt + 1) * 128, c * CW:(c + 1) * CW],
                in_=o_s)

        # ---- filler machinery ----
        filler = deque()

        def drain(n):
            for _ in range(min(n, len(filler))):
                filler.popleft()()

        def drain_all():
            while filler:
                filler.popleft()()

        # ---- prelude: chunk 0 projections ----
        load_xt(0)
        for pg in range(2):
            qk_proj("k", pg, 0)
            qk_proj("q", pg, 0)
        for jj in range(4):
            v_unit(0, jj)

        # ---- main chunk-outer loop ----
        for c in range(NC_):
            if c + 1 < NC_:
                filler.append(partial(load_xt, c + 1))
                for pg in range(2):
                    filler.append(partial(qk_proj, "k", pg, c + 1))
                    filler.append(partial(qk_proj, "q", pg, c + 1))
                for jj in range(4 * (c + 1), 4 * (c + 1) + 4):
                    filler.append(partial(v_unit, c + 1, jj))
            if c >= 1:
                filler.append(partial(oproj_yin, c - 1))
                for dt in range(8):
                    filler.append(partial(oproj_dt, c - 1, dt))

            for P in range(4):
                hA, hB = 2 * P, 2 * P + 1
                qt_c = q_tiles.pop((P, c))
                pOA = psp.tile([128, CW], F32, tag="pO", bufs=2,
                               name=f"pOA{P}_{c}")
                pOB = psp.tile([128, CW], F32, tag="pO", bufs=2,
                               name=f"pOB{P}_{c}")
                for g in range(2 * c + 2):
                    pSg = psp.tile([128, 4 * CW], F32, tag="pSg", bufs=1,
                                   name="pSg")
                    for idx, rows in ((0, slice(0, 64)), (1, slice(64, 128))):
                        for e in (0, 1):
                            j = 2 * g + e
                            nc.tensor.matmul(
                                pSg[:, (2 * idx + e) * CW:
                                    (2 * idx + e + 1) * CW],
                                kt[P][j // 4][rows,
                                              (j % 4) * 128:(j % 4 + 1) * 128],
                                qt_c[rows, :], start=True, stop=True)
                    ptA = ptp.tile([128, 2 * CW], F32R, tag="pt", name="ptA")
                    nc.scalar.activation(ptA, pSg[:, 0:2 * CW], AF.Exp,
                                         scale=0.125)
                    ptB = ptp.tile([128, 2 * CW], F32R, tag="pt", name="ptB")
                    nc.scalar.activation(ptB, pSg[:, 2 * CW:4 * CW], AF.Exp,
                                         scale=0.125)
                    if g == 2 * c:
                        nc.vector.tensor_mul(ptA[:, 0:128], ptA[:, 0:128],
                                             tri)
                        nc.vector.tensor_mul(ptA[:, CW:CW + 256],
                                             ptA[:, CW:CW + 256], mB)
                        nc.gpsimd.tensor_mul(ptB[:, 0:128], ptB[:, 0:128],
                                             tri)
                        nc.gpsimd.tensor_mul(ptB[:, CW:CW + 256],
                                             ptB[:, CW:CW + 256], mB)
                    elif g == 2 * c + 1:
                        nc.vector.tensor_mul(ptA[:, 0:384], ptA[:, 0:384],
                                             mC)
                        nc.vector.tensor_mul(ptA[:, CW:CW + 512],
                                             ptA[:, CW:CW + 512], mD)
                        nc.gpsimd.tensor_mul(ptB[:, 0:384], ptB[:, 0:384],
                                             mC)
                        nc.gpsimd.tensor_mul(ptB[:, CW:CW + 512],
                                             ptB[:, CW:CW + 512], mD)
                    drain(2)
                    for pt_t, h, pO_t in ((ptA, hA, pOA), (ptB, hB, pOB)):
                        for e in (0, 1):
                            j = 2 * g + e
                            nc.tensor.matmul(
                                pO_t[0:65, :],
                                vpall[:, j, h * 65:(h + 1) * 65],
                                pt_t[:, e * CW:(e + 1) * CW],
                                start=(j == 0), stop=(j == 4 * c + 3))

                # output stage for this (pair, chunk)
                for h, pO_t in ((hA, pOA), (hB, pOB)):
                    den = osp.tile([1, CW], F32, tag="den")
                    nc.vector.tensor_copy(den, pO_t[64:65, :])
                    rec = osp.tile([1, CW], F32, tag="rec")
                    nc.vector.reciprocal_approx_fast(rec, den)
                    rb = osp.tile([64, CW], F32, tag="rb")
                    nc.gpsimd.partition_broadcast(rb, rec)
                    ys = osp.tile([64, CW], F32R, tag="ys")
                    nc.vector.tensor_mul(ys, pO_t[0:64, :], rb)
                    nc.gpsimd.dma_start(
                        out=yd[h * 64:(h + 1) * 64, c * CW:(c + 1) * CW],
                        in_=ys)
                drain(1)
            drain_all()

        oproj_yin(3)
        for dt in range(8):
            oproj_dt(3, dt)

    nc.finalize()
    return nc


def _prep_inputs(x, q_proj, k_proj, v_proj, o_proj):
    pos = np.arange(S, dtype=np.float64)
    inv = THETA ** (-np.arange(0, DK, 2, dtype=np.float64) / DK)   # [32]
    ang = inv[:, None] * pos[None, :]                              # [32, S]
    cos_big = np.tile(np.cos(ang), (4, 1)).astype(np.float32)
    sin_big = np.tile(np.sin(ang), (4, 1)).astype(np.float32)

    in_maps = []
    for core in range(NCORES):
        b, g = core // 2, core % 2
        heads = [g * HPC + i for i in range(HPC)]
        rows_x1 = [h * DK + 2 * e for h in heads for e in range(32)]
        rows_x2 = [h * DK + 2 * e + 1 for h in heads for e in range(32)]
        perm = rows_x1 + rows_x2
        nat = [h * DK + d_ for h in heads for d_ in range(DK)]
        in_maps.append({
            "XT": np.ascontiguousarray(x[b].T, dtype=np.float32),
            "WQ": np.ascontiguousarray(q_proj[perm, :].T, dtype=np.float32),
            "WK": np.ascontiguousarray(k_proj[perm, :].T, dtype=np.float32),
            "WV": np.ascontiguousarray(v_proj[nat, :].T, dtype=np.float32),
            "OC": np.ascontiguousarray(o_proj[:, nat].T, dtype=np.float32),
            "COS": cos_big,
            "SIN": sin_big,
        })
    return in_maps


def _run(in_maps, **kw):
    if "nc" not in _CACHE:
        _CACHE["nc"] = _build_nc()
    return run_bass_kernel_spmd(_CACHE["nc"], in_maps,
                                core_ids=list(range(NCORES)), **kw)


def kernel(x, q_proj, k_proj, v_proj, o_proj):
    x = np.asarray(x, dtype=np.float32)
    in_maps = _prep_inputs(x,
                           np.asarray(q_proj, dtype=np.float32),
                           np.asarray(k_proj, dtype=np.float32),
                           np.asarray(v_proj, dtype=np.float32),
                           np.asarray(o_proj, dtype=np.float32))
    res = _run(in_maps)
    B = x.shape[0]
    out = np.empty((B, S, D), dtype=np.float32)
    for b in range(B):
        ot = res.results[2 * b]["OT"] + res.results[2 * b + 1]["OT"]
        out[b] = ot.T
    return out
